# revision 19
# baseline (speedup 1.0000x reference)
"""Trainium2 Bass kernel for nn_Apply_Mask (topk_masking).

Reference semantics, per (batch, channel) slice of shape 32x32:
  - find argmax location (mh, mw)
  - build clipped 5x5 box around it; S = 1 - box
  - lam = 1024 / sum(S)
  - out = T != 0 ? x * S * lam : x

Sharding: embarrassingly data-parallel over the 32768 (b*c) slices;
core i takes slices [4096*i, 4096*(i+1)).

Per-core layout: partition p holds 32 slices [32p, 32p+32) along the free
dim; tile t = slice 32p+t at free offset t*1024.

Engine split:
  DVE    : exact f32 argmax (max8/find_index8), iota compares, per-slice
           scalar math, apply out = (q + a) * x via scalar_tensor_tensor
  GpSimd : mask subtracts + outer products q = row_nb (x) col_in
           (Pool ISA only allows arithmetic TT ops; compares stay on DVE)
  ScalarE: f32 -> bf16 downcast of the output before DMA
  DMA    : f32 in (16 MiB), bf16 out (8 MiB)

Processing is split into NGROUP groups of tiles so group g's mask/apply
work overlaps group g+1's argmax.
"""
import sys

for _p in ("/opt/trn_rl_repo",):
    if _p not in sys.path:
        sys.path.insert(0, _p)

import numpy as np

import concourse.bass as bass
import concourse.tile as tile
from concourse import bacc, mybir
from concourse.bass_utils import run_bass_kernel_spmd

P = 128          # partitions
NT = 32          # tiles (slices) per partition
H = W = 32
HW = H * W
N_CORES = 8
SLICES_PER_CORE = P * NT  # 4096

OUT_BF16 = True   # downcast output to bf16 on ScalarE before DMA (rel err ~2e-3)
KQ = 2            # tiles per outer-product instruction
NGROUP = 4        # tile groups for pipelining
POOL_APPLY = 3    # applies per group offloaded to GpSimd (tensor_scalar + TT)
# input DMA chunk sizes in tiles (first chunks small to start compute early)
IN_CHUNKS = [1, 1, 2, 4, 4, 4, 4, 4, 4, 4]
OUT_CH_T = 4      # tiles per output chunk

f32 = mybir.dt.float32
bf16 = mybir.dt.bfloat16
u16 = mybir.dt.uint16
Alu = mybir.AluOpType
Act = mybir.ActivationFunctionType

_cached = {}


def _build(half: int):
    assert sum(IN_CHUNKS) == NT
    odt = bf16 if OUT_BF16 else f32
    GT = NT // NGROUP      # tiles per group

    nc = bacc.Bacc("TRN2", target_bir_lowering=False, debug=False,
                   num_devices=N_CORES)
    x_in = nc.dram_tensor("x", [P, NT * HW], f32, kind="ExternalInput").ap()
    sel_in = nc.dram_tensor("sel", [P, NT], f32, kind="ExternalInput").ap()
    io_in = nc.dram_tensor("io32", [P, 32], f32, kind="ExternalInput").ap()
    out_d = nc.dram_tensor("out", [P, NT * HW], odt, kind="ExternalOutput").ap()

    with tile.TileContext(nc) as tc:
        from contextlib import ExitStack
        with ExitStack() as ctx:
            xpool = ctx.enter_context(tc.tile_pool(name="xp", bufs=1))
            mid = ctx.enter_context(tc.tile_pool(name="mid", bufs=1))
            small = ctx.enter_context(tc.tile_pool(name="small", bufs=1))
            qpool = ctx.enter_context(tc.tile_pool(name="qp", bufs=3))
            ypool = ctx.enter_context(tc.tile_pool(name="yp", bufs=2))
            opool = ctx.enter_context(tc.tile_pool(name="op", bufs=2))

            # ---- input DMA chunks ----
            xc = []          # list of (tile, first_tile, ntiles)
            t0 = 0
            for ci, ct in enumerate(IN_CHUNKS):
                t_ = xpool.tile([P, ct * HW], f32, name=f"x{ci}", tag=f"x{ci}")
                nc.sync.dma_start(t_[:], x_in[:, t0 * HW:(t0 + ct) * HW])
                xc.append((t_, t0, ct))
                t0 += ct

            def x_tile(t):
                for t_, lo, ct in xc:
                    if lo <= t < lo + ct:
                        return t_[:, (t - lo) * HW:(t - lo + 1) * HW]
                raise KeyError(t)

            selp = small.tile([P, NT], f32)
            nc.sync.dma_start(selp[:], sel_in)
            io32 = small.tile([P, 32], f32)
            nc.sync.dma_start(io32[:], io_in)

            max8 = mid.tile([P, NT, 8], f32)
            idx8 = mid.tile([P, NT, 8], u16)
            col_in = mid.tile([P, NT, W], f32)
            col_gt = mid.tile([P, NT, W], f32)
            row_nb = mid.tile([P, NT, H], f32)
            row_gt = mid.tile([P, NT, H], f32)
            io_b = io32[:, None, :]

            def smalls(name, n=NGROUP):
                return [small.tile([P, GT], f32, name=f"{name}{g}", tag=f"{name}{g}")
                        for g in range(n)]

            idx_u = [small.tile([P, GT], u16, name=f"idxu{g}", tag=f"idxu{g}")
                     for g in range(NGROUP)]
            mh_u = [small.tile([P, GT], u16, name=f"mhu{g}", tag=f"mhu{g}")
                    for g in range(NGROUP)]
            mw_u = [small.tile([P, GT], u16, name=f"mwu{g}", tag=f"mwu{g}")
                    for g in range(NGROUP)]
            mh = smalls("mh"); mw = smalls("mw")
            h1 = smalls("h1"); h2 = smalls("h2"); w1 = smalls("w1"); w2 = smalls("w2")
            rl = smalls("rl"); cl1 = smalls("cl1"); area = smalls("area")
            denom = smalls("denom"); recip = smalls("recip"); lam1 = smalls("lam1")
            a_t = smalls("a"); nb_t = smalls("nb")

            o_cs = []

            for g in range(NGROUP):
                gl = g * GT          # first tile of group
                gsl = slice(gl, gl + GT)

                # ---- argmax: all max8s, then all find_index8s ----
                for t in range(gl, gl + GT):
                    nc.vector.max(max8[:, t], x_tile(t))
                for t in range(gl, gl + GT):
                    nc.vector.max_index(idx8[:, t], max8[:, t], x_tile(t))

                # ---- per-slice scalar math ----
                nc.vector.tensor_copy(idx_u[g][:], idx8[:, gsl, 0])
                nc.vector.tensor_scalar(mh_u[g][:], idx_u[g][:], 5, None, Alu.logical_shift_right)
                nc.vector.tensor_scalar(mw_u[g][:], idx_u[g][:], 31, None, Alu.bitwise_and)
                nc.vector.tensor_copy(mh[g][:], mh_u[g][:])
                nc.vector.tensor_copy(mw[g][:], mw_u[g][:])
                nc.vector.tensor_scalar(h1[g][:], mh[g][:], float(half), 0.0, Alu.subtract, Alu.max)
                nc.vector.tensor_scalar(h2[g][:], mh[g][:], float(half), float(H - 1), Alu.add, Alu.min)
                nc.vector.tensor_scalar(w1[g][:], mw[g][:], float(half), 0.0, Alu.subtract, Alu.max)
                nc.vector.tensor_scalar(w2[g][:], mw[g][:], float(half), float(W - 1), Alu.add, Alu.min)
                nc.vector.tensor_tensor(rl[g][:], h2[g][:], h1[g][:], Alu.subtract)
                nc.vector.tensor_tensor(cl1[g][:], w2[g][:], w1[g][:], Alu.subtract)
                nc.vector.tensor_scalar(cl1[g][:], cl1[g][:], 1.0, None, Alu.add)
                nc.vector.scalar_tensor_tensor(area[g][:], rl[g][:], 1.0, cl1[g][:], Alu.add, Alu.mult)
                nc.vector.tensor_scalar(denom[g][:], area[g][:], -1.0, float(HW), Alu.mult, Alu.add)
                nc.vector.reciprocal(recip[g][:], denom[g][:])
                nc.vector.tensor_scalar(lam1[g][:], recip[g][:], float(HW), -1.0, Alu.mult, Alu.add)
                nc.vector.scalar_tensor_tensor(a_t[g][:], lam1[g][:], 0.0, selp[:, gsl], Alu.add, Alu.mult)
                nc.vector.tensor_scalar(a_t[g][:], a_t[g][:], 1.0, None, Alu.add)
                nc.vector.scalar_tensor_tensor(nb_t[g][:], a_t[g][:], 1.0, selp[:, gsl], Alu.subtract, Alu.add)
                nc.vector.tensor_scalar(nb_t[g][:], nb_t[g][:], -1.0, None, Alu.mult)

                # ---- masks (compares on DVE, sub on Pool, row*-b on DVE) ----
                iog = io_b.broadcast_to([P, GT, 32])
                nc.vector.tensor_tensor(col_in[:, gsl], iog, w1[g][:, :, None].broadcast_to([P, GT, W]), Alu.is_ge)
                nc.vector.tensor_tensor(col_gt[:, gsl], iog, w2[g][:, :, None].broadcast_to([P, GT, W]), Alu.is_gt)
                nc.gpsimd.tensor_tensor(col_in[:, gsl], col_in[:, gsl], col_gt[:, gsl], Alu.subtract)
                nc.vector.tensor_tensor(row_nb[:, gsl], iog, h1[g][:, :, None].broadcast_to([P, GT, H]), Alu.is_ge)
                nc.vector.tensor_tensor(row_gt[:, gsl], iog, h2[g][:, :, None].broadcast_to([P, GT, H]), Alu.is_gt)
                nc.gpsimd.tensor_tensor(row_nb[:, gsl], row_nb[:, gsl], row_gt[:, gsl], Alu.subtract)
                nc.vector.tensor_tensor(row_nb[:, gsl], row_nb[:, gsl], nb_t[g][:, :, None].broadcast_to([P, GT, H]), Alu.mult)

                # ---- outer products on Pool, KQ tiles per instruction ----
                qb = {}
                for b_ in range(GT // KQ):
                    tb = gl + b_ * KQ
                    q = qpool.tile([P, KQ, H, W], f32, name=f"q{tb}", tag="q")
                    nc.gpsimd.tensor_tensor(
                        q[:],
                        row_nb[:, tb:tb + KQ, :, None].broadcast_to([P, KQ, H, W]),
                        col_in[:, tb:tb + KQ, None, :].broadcast_to([P, KQ, H, W]),
                        Alu.mult,
                    )
                    qb[tb] = q

                # ---- apply -> ScalarE downcast chunks ----
                # Last POOL_APPLY tiles of each group run on GpSimd
                # (tensor_scalar add + TT mult) to offload the DVE.
                for c in range(GT // OUT_CH_T):
                    c0 = gl + c * OUT_CH_T
                    o_c = opool.tile([P, OUT_CH_T * HW], odt, name=f"o{c0}", tag="oc")
                    for j in range(OUT_CH_T):
                        t = c0 + j
                        q = qb[(t // KQ) * KQ]
                        of = ypool.tile([P, H, W], f32, name=f"of{t}", tag="of")
                        if t - gl >= GT - POOL_APPLY:
                            t1 = ypool.tile([P, H, W], f32, name=f"t1_{t}", tag="t1")
                            nc.gpsimd.tensor_scalar(
                                t1[:], q[:, t % KQ], a_t[g][:, t - gl, None],
                                None, Alu.add)
                            nc.gpsimd.tensor_tensor(
                                of[:], t1[:],
                                x_tile(t).rearrange("p (h w) -> p h w", h=H, w=W),
                                Alu.mult)
                        else:
                            nc.vector.scalar_tensor_tensor(
                                of[:], q[:, t % KQ], a_t[g][:, t - gl, None],
                                x_tile(t).rearrange("p (h w) -> p h w", h=H, w=W),
                                Alu.add, Alu.mult,
                            )
                        nc.scalar.copy(
                            o_c[:, j * HW:(j + 1) * HW],
                            of.rearrange("p h w -> p (h w)"))
                    o_cs.append((c0, o_c))
                    nc.sync.dma_start(
                        out_d[:, c0 * HW:(c0 + OUT_CH_T) * HW], o_c[:])

    nc.compile()
    return nc


def _get_nc(half: int):
    if half not in _cached:
        _cached[half] = _build(half)
    return _cached[half]


def _shard_inputs(x, T):
    xf = np.ascontiguousarray(x, dtype=np.float32).reshape(-1, HW)   # [32768, 1024]
    sel = (np.asarray(T).reshape(-1) != 0).astype(np.float32)        # [32768]
    io32 = np.tile(np.arange(32, dtype=np.float32), (P, 1))
    in_maps = []
    for i in range(N_CORES):
        lo = i * SLICES_PER_CORE
        hi = lo + SLICES_PER_CORE
        in_maps.append({
            "x": np.ascontiguousarray(xf[lo:hi].reshape(P, NT * HW)),
            "sel": np.ascontiguousarray(sel[lo:hi].reshape(P, NT)),
            "io32": io32,
        })
    return in_maps


def run(inputs, trace=False, **kw):
    x = inputs["x"]
    T = inputs["T"]
    drop_block = int(np.asarray(inputs["drop_block"]))
    half = drop_block // 2
    b, c, h, w = x.shape
    assert (h, w) == (H, W) and b * c == N_CORES * SLICES_PER_CORE, \
        f"kernel hardcoded for (128,256,32,32); got {x.shape}"

    nc = _get_nc(half)
    in_maps = _shard_inputs(x, T)
    res = run_bass_kernel_spmd(nc, in_maps, core_ids=list(range(N_CORES)),
                               trace=trace, **kw)
    parts = [np.asarray(res.results[i]["out"]).astype(np.float32)
              .reshape(SLICES_PER_CORE, HW)
             for i in range(N_CORES)]
    out = np.concatenate(parts, axis=0).reshape(b, c, h, w)
    return out, res


def kernel(**inputs) -> np.ndarray:
    out, _ = run(inputs, trace=False)
    return out


# revision 20
# speedup vs baseline: 2.2338x; 2.2338x over previous
"""Trainium2 Bass kernel for nn_Apply_Mask (topk_masking).

Reference semantics, per (batch, channel) slice of shape 32x32:
  - find argmax location (mh, mw)
  - build clipped 5x5 box around it; S = 1 - box
  - lam = 1024 / sum(S)
  - out = T != 0 ? x * S * lam : x

Sharding: embarrassingly data-parallel over the 32768 (b*c) slices;
core i takes slices [4096*i, 4096*(i+1)).

Per-core layout: partition p holds 32 slices [32p, 32p+32) along the free
dim; tile t = slice 32p+t at free offset t*1024.

Engine split:
  DVE    : exact f32 argmax (max8/find_index8), iota compares, per-slice
           scalar math, apply out = (q + a) * x via scalar_tensor_tensor
  GpSimd : mask subtracts + outer products q = row_nb (x) col_in
           (Pool ISA only allows arithmetic TT ops; compares stay on DVE)
  ScalarE: f32 -> bf16 downcast of the output before DMA
  DMA    : f32 in (16 MiB), bf16 out (8 MiB)

Processing is split into NGROUP groups of tiles so group g's mask/apply
work overlaps group g+1's argmax.
"""
import sys

for _p in ("/opt/trn_rl_repo",):
    if _p not in sys.path:
        sys.path.insert(0, _p)

import numpy as np

import concourse.bass as bass
import concourse.tile as tile
from concourse import bacc, mybir
from concourse.bass_utils import run_bass_kernel_spmd

P = 128          # partitions
NT = 32          # tiles (slices) per partition
H = W = 32
HW = H * W
N_CORES = 8
SLICES_PER_CORE = P * NT  # 4096

OUT_BF16 = True   # downcast output to bf16 on ScalarE before DMA (rel err ~2e-3)
KQ = 2            # tiles per outer-product instruction
NGROUP = 4        # tile groups for pipelining
POOL_APPLY = 0    # applies per group offloaded to GpSimd (slow: TS=18us/op on Pool)
# input DMA chunk sizes in tiles (first chunks small to start compute early)
IN_CHUNKS = [1, 1, 2, 4, 4, 4, 4, 4, 4, 4]
OUT_CH_T = 4      # tiles per output chunk

f32 = mybir.dt.float32
bf16 = mybir.dt.bfloat16
u16 = mybir.dt.uint16
Alu = mybir.AluOpType
Act = mybir.ActivationFunctionType

_cached = {}


def _build(half: int):
    assert sum(IN_CHUNKS) == NT
    odt = bf16 if OUT_BF16 else f32
    GT = NT // NGROUP      # tiles per group

    nc = bacc.Bacc("TRN2", target_bir_lowering=False, debug=False,
                   num_devices=N_CORES)
    x_in = nc.dram_tensor("x", [P, NT * HW], f32, kind="ExternalInput").ap()
    sel_in = nc.dram_tensor("sel", [P, NT], f32, kind="ExternalInput").ap()
    io_in = nc.dram_tensor("io32", [P, 32], f32, kind="ExternalInput").ap()
    out_d = nc.dram_tensor("out", [P, NT * HW], odt, kind="ExternalOutput").ap()

    with tile.TileContext(nc) as tc:
        from contextlib import ExitStack
        with ExitStack() as ctx:
            xpool = ctx.enter_context(tc.tile_pool(name="xp", bufs=1))
            mid = ctx.enter_context(tc.tile_pool(name="mid", bufs=1))
            small = ctx.enter_context(tc.tile_pool(name="small", bufs=1))
            qpool = ctx.enter_context(tc.tile_pool(name="qp", bufs=3))
            ypool = ctx.enter_context(tc.tile_pool(name="yp", bufs=2))
            opool = ctx.enter_context(tc.tile_pool(name="op", bufs=2))

            # ---- input DMA chunks ----
            xc = []          # list of (tile, first_tile, ntiles)
            t0 = 0
            for ci, ct in enumerate(IN_CHUNKS):
                t_ = xpool.tile([P, ct * HW], f32, name=f"x{ci}", tag=f"x{ci}")
                nc.sync.dma_start(t_[:], x_in[:, t0 * HW:(t0 + ct) * HW])
                xc.append((t_, t0, ct))
                t0 += ct

            def x_tile(t):
                for t_, lo, ct in xc:
                    if lo <= t < lo + ct:
                        return t_[:, (t - lo) * HW:(t - lo + 1) * HW]
                raise KeyError(t)

            selp = small.tile([P, NT], f32)
            nc.sync.dma_start(selp[:], sel_in)
            io32 = small.tile([P, 32], f32)
            nc.sync.dma_start(io32[:], io_in)

            max8 = mid.tile([P, NT, 8], f32)
            idx8 = mid.tile([P, NT, 8], u16)
            col_in = mid.tile([P, NT, W], f32)
            col_gt = mid.tile([P, NT, W], f32)
            row_nb = mid.tile([P, NT, H], f32)
            row_gt = mid.tile([P, NT, H], f32)
            io_b = io32[:, None, :]

            def smalls(name, n=NGROUP):
                return [small.tile([P, GT], f32, name=f"{name}{g}", tag=f"{name}{g}")
                        for g in range(n)]

            idx_u = [small.tile([P, GT], u16, name=f"idxu{g}", tag=f"idxu{g}")
                     for g in range(NGROUP)]
            mh_u = [small.tile([P, GT], u16, name=f"mhu{g}", tag=f"mhu{g}")
                    for g in range(NGROUP)]
            mw_u = [small.tile([P, GT], u16, name=f"mwu{g}", tag=f"mwu{g}")
                    for g in range(NGROUP)]
            mh = smalls("mh"); mw = smalls("mw")
            h1 = smalls("h1"); h2 = smalls("h2"); w1 = smalls("w1"); w2 = smalls("w2")
            rl = smalls("rl"); cl1 = smalls("cl1"); area = smalls("area")
            denom = smalls("denom"); recip = smalls("recip"); lam1 = smalls("lam1")
            a_t = smalls("a"); nb_t = smalls("nb")

            o_cs = []

            for g in range(NGROUP):
                gl = g * GT          # first tile of group
                gsl = slice(gl, gl + GT)

                # ---- argmax: all max8s, then all find_index8s ----
                for t in range(gl, gl + GT):
                    nc.vector.max(max8[:, t], x_tile(t))
                for t in range(gl, gl + GT):
                    nc.vector.max_index(idx8[:, t], max8[:, t], x_tile(t))

                # ---- per-slice scalar math ----
                nc.vector.tensor_copy(idx_u[g][:], idx8[:, gsl, 0])
                nc.vector.tensor_scalar(mh_u[g][:], idx_u[g][:], 5, None, Alu.logical_shift_right)
                nc.vector.tensor_scalar(mw_u[g][:], idx_u[g][:], 31, None, Alu.bitwise_and)
                nc.vector.tensor_copy(mh[g][:], mh_u[g][:])
                nc.vector.tensor_copy(mw[g][:], mw_u[g][:])
                nc.vector.tensor_scalar(h1[g][:], mh[g][:], float(half), 0.0, Alu.subtract, Alu.max)
                nc.vector.tensor_scalar(h2[g][:], mh[g][:], float(half), float(H - 1), Alu.add, Alu.min)
                nc.vector.tensor_scalar(w1[g][:], mw[g][:], float(half), 0.0, Alu.subtract, Alu.max)
                nc.vector.tensor_scalar(w2[g][:], mw[g][:], float(half), float(W - 1), Alu.add, Alu.min)
                nc.vector.tensor_tensor(rl[g][:], h2[g][:], h1[g][:], Alu.subtract)
                nc.vector.tensor_tensor(cl1[g][:], w2[g][:], w1[g][:], Alu.subtract)
                nc.vector.tensor_scalar(cl1[g][:], cl1[g][:], 1.0, None, Alu.add)
                nc.vector.scalar_tensor_tensor(area[g][:], rl[g][:], 1.0, cl1[g][:], Alu.add, Alu.mult)
                nc.vector.tensor_scalar(denom[g][:], area[g][:], -1.0, float(HW), Alu.mult, Alu.add)
                nc.vector.reciprocal(recip[g][:], denom[g][:])
                nc.vector.tensor_scalar(lam1[g][:], recip[g][:], float(HW), -1.0, Alu.mult, Alu.add)
                nc.vector.scalar_tensor_tensor(a_t[g][:], lam1[g][:], 0.0, selp[:, gsl], Alu.add, Alu.mult)
                nc.vector.tensor_scalar(a_t[g][:], a_t[g][:], 1.0, None, Alu.add)
                nc.vector.scalar_tensor_tensor(nb_t[g][:], a_t[g][:], 1.0, selp[:, gsl], Alu.subtract, Alu.add)
                nc.vector.tensor_scalar(nb_t[g][:], nb_t[g][:], -1.0, None, Alu.mult)

                # ---- masks (compares on DVE, sub on Pool, row*-b on DVE) ----
                iog = io_b.broadcast_to([P, GT, 32])
                nc.vector.tensor_tensor(col_in[:, gsl], iog, w1[g][:, :, None].broadcast_to([P, GT, W]), Alu.is_ge)
                nc.vector.tensor_tensor(col_gt[:, gsl], iog, w2[g][:, :, None].broadcast_to([P, GT, W]), Alu.is_gt)
                nc.gpsimd.tensor_tensor(col_in[:, gsl], col_in[:, gsl], col_gt[:, gsl], Alu.subtract)
                nc.vector.tensor_tensor(row_nb[:, gsl], iog, h1[g][:, :, None].broadcast_to([P, GT, H]), Alu.is_ge)
                nc.vector.tensor_tensor(row_gt[:, gsl], iog, h2[g][:, :, None].broadcast_to([P, GT, H]), Alu.is_gt)
                nc.gpsimd.tensor_tensor(row_nb[:, gsl], row_nb[:, gsl], row_gt[:, gsl], Alu.subtract)
                nc.vector.tensor_tensor(row_nb[:, gsl], row_nb[:, gsl], nb_t[g][:, :, None].broadcast_to([P, GT, H]), Alu.mult)

                # ---- outer products on Pool, KQ tiles per instruction ----
                qb = {}
                for b_ in range(GT // KQ):
                    tb = gl + b_ * KQ
                    q = qpool.tile([P, KQ, H, W], f32, name=f"q{tb}", tag="q")
                    nc.gpsimd.tensor_tensor(
                        q[:],
                        row_nb[:, tb:tb + KQ, :, None].broadcast_to([P, KQ, H, W]),
                        col_in[:, tb:tb + KQ, None, :].broadcast_to([P, KQ, H, W]),
                        Alu.mult,
                    )
                    qb[tb] = q

                # ---- apply -> ScalarE downcast chunks ----
                # Last POOL_APPLY tiles of each group run on GpSimd
                # (tensor_scalar add + TT mult) to offload the DVE.
                for c in range(GT // OUT_CH_T):
                    c0 = gl + c * OUT_CH_T
                    o_c = opool.tile([P, OUT_CH_T * HW], odt, name=f"o{c0}", tag="oc")
                    for j in range(OUT_CH_T):
                        t = c0 + j
                        q = qb[(t // KQ) * KQ]
                        of = ypool.tile([P, H, W], f32, name=f"of{t}", tag="of")
                        if t - gl >= GT - POOL_APPLY:
                            t1 = ypool.tile([P, H, W], f32, name=f"t1_{t}", tag="t1")
                            nc.gpsimd.tensor_scalar(
                                t1[:], q[:, t % KQ], a_t[g][:, t - gl, None],
                                None, Alu.add)
                            nc.gpsimd.tensor_tensor(
                                of[:], t1[:],
                                x_tile(t).rearrange("p (h w) -> p h w", h=H, w=W),
                                Alu.mult)
                        else:
                            nc.vector.scalar_tensor_tensor(
                                of[:], q[:, t % KQ], a_t[g][:, t - gl, None],
                                x_tile(t).rearrange("p (h w) -> p h w", h=H, w=W),
                                Alu.add, Alu.mult,
                            )
                        nc.scalar.copy(
                            o_c[:, j * HW:(j + 1) * HW],
                            of.rearrange("p h w -> p (h w)"))
                    o_cs.append((c0, o_c))
                    nc.sync.dma_start(
                        out_d[:, c0 * HW:(c0 + OUT_CH_T) * HW], o_c[:])

    nc.compile()
    return nc


def _get_nc(half: int):
    if half not in _cached:
        _cached[half] = _build(half)
    return _cached[half]


def _shard_inputs(x, T):
    xf = np.ascontiguousarray(x, dtype=np.float32).reshape(-1, HW)   # [32768, 1024]
    sel = (np.asarray(T).reshape(-1) != 0).astype(np.float32)        # [32768]
    io32 = np.tile(np.arange(32, dtype=np.float32), (P, 1))
    in_maps = []
    for i in range(N_CORES):
        lo = i * SLICES_PER_CORE
        hi = lo + SLICES_PER_CORE
        in_maps.append({
            "x": np.ascontiguousarray(xf[lo:hi].reshape(P, NT * HW)),
            "sel": np.ascontiguousarray(sel[lo:hi].reshape(P, NT)),
            "io32": io32,
        })
    return in_maps


def run(inputs, trace=False, **kw):
    x = inputs["x"]
    T = inputs["T"]
    drop_block = int(np.asarray(inputs["drop_block"]))
    half = drop_block // 2
    b, c, h, w = x.shape
    assert (h, w) == (H, W) and b * c == N_CORES * SLICES_PER_CORE, \
        f"kernel hardcoded for (128,256,32,32); got {x.shape}"

    nc = _get_nc(half)
    in_maps = _shard_inputs(x, T)
    res = run_bass_kernel_spmd(nc, in_maps, core_ids=list(range(N_CORES)),
                               trace=trace, **kw)
    parts = [np.asarray(res.results[i]["out"]).astype(np.float32)
              .reshape(SLICES_PER_CORE, HW)
             for i in range(N_CORES)]
    out = np.concatenate(parts, axis=0).reshape(b, c, h, w)
    return out, res


def kernel(**inputs) -> np.ndarray:
    out, _ = run(inputs, trace=False)
    return out
